# revision 27
# baseline (speedup 1.0000x reference)
"""Trainium2 Bass kernel for MultiHeadAttention (B=2, L=2048, D=1024, H=16, dqk=64).

Returns (o, attn) like the reference:
  o    [2, 2048, 1024] f32  (LayerNorm(residual + fc(attention_out)))
  attn [2, 16, 2048, 2048] f32

Sharding: 8 cores; core c covers batch b=c//4 and head-group g=c%4 (heads 4g..4g+3).
Stage 1 (per core): projections (bf16 matmuls, fp32 PSUM accumulate), scores both
orientations, softmax via exp(S/8 - logZ) with Z from a ones-column in the AV matmul,
attention output, partial FC.  Stage 2 (row-sharded): sum the 4 group-partial FC
outputs, add residual, LayerNorm.
"""

import numpy as np
import ml_dtypes

import concourse.bass as bass
import concourse.tile as tile
from concourse import bacc, mybir
from concourse.bass_utils import run_bass_kernel_spmd

BF16 = ml_dtypes.bfloat16
B, L, D = 2, 2048, 1024
H, DQK = 16, 64
G = 4              # head groups
HPG = H // G       # heads per group = 4
HD = HPG * DQK     # 256 per-core head dims
NCORES = 8
P = 128
NDT = D // P       # 8 D-tiles
NLT = L // P       # 16 L-tiles
INV_TEMP = 1.0 / (DQK ** 0.5)
EPS = 1e-6

F32 = mybir.dt.float32
BF = mybir.dt.bfloat16


def _build_stage1():
    nc = bacc.Bacc("TRN2", target_bir_lowering=False, debug=False,
                   num_devices=NCORES)
    qT = nc.dram_tensor("qT", [D, L], BF, kind="ExternalInput").ap()
    kT = nc.dram_tensor("kT", [D, L], BF, kind="ExternalInput").ap()
    vT = nc.dram_tensor("vT", [D, L], BF, kind="ExternalInput").ap()
    wq = nc.dram_tensor("wq", [D, HD], BF, kind="ExternalInput").ap()
    wk = nc.dram_tensor("wk", [D, HD], BF, kind="ExternalInput").ap()
    wv = nc.dram_tensor("wv", [D, HD], BF, kind="ExternalInput").ap()
    wfc = nc.dram_tensor("wfc", [HD, D], BF, kind="ExternalInput").ap()
    attn_out = nc.dram_tensor("attn_part", [HPG, L, L], F32,
                              kind="ExternalOutput").ap()
    fc_out = nc.dram_tensor("fc_part", [L, D], BF, kind="ExternalOutput").ap()

    with tile.TileContext(nc) as tc:
        _stage1_body(tc, qT, kT, vT, wq, wk, wv, wfc, attn_out, fc_out)
    nc.compile()
    return nc


def _stage1_body(tc, qT, kT, vT, wq, wk, wv, wfc, attn_out, fc_out):
    nc = tc.nc
    import contextlib
    ctx = contextlib.ExitStack()
    with ctx:
        stream = ctx.enter_context(tc.tile_pool(name="stream", bufs=2))
        vstream = ctx.enter_context(tc.tile_pool(name="vstream", bufs=3))
        wp = ctx.enter_context(tc.tile_pool(name="wp", bufs=1))
        persist = ctx.enter_context(tc.tile_pool(name="persist", bufs=1))
        etp = ctx.enter_context(tc.tile_pool(name="etp", bufs=20))
        atp = ctx.enter_context(tc.tile_pool(name="atp", bufs=3))
        fcp = ctx.enter_context(tc.tile_pool(name="fcp", bufs=2))
        zp = ctx.enter_context(tc.tile_pool(name="zp", bufs=1))
        ps1 = ctx.enter_context(tc.tile_pool(name="ps1", bufs=2, space="PSUM"))
        psO = ctx.enter_context(tc.tile_pool(name="psO", bufs=1, space="PSUM"))

        # ---- weights ----
        wq_sb = wp.tile([P, NDT, HD], BF, tag="wq")
        wk_sb = wp.tile([P, NDT, HD], BF, tag="wk")
        wv_sb = wp.tile([P, NDT, HD], BF, tag="wv")
        wfc_sb = wp.tile([P, HD // P, D], BF, tag="wfc")
        nc.sync.dma_start(wq_sb[:], wq.rearrange("(t p) n -> p t n", p=P))
        nc.sync.dma_start(wk_sb[:], wk.rearrange("(t p) n -> p t n", p=P))


        # ---- q/k projections, 1024-wide L chunks ----
        qhT = [persist.tile([P, L], BF, tag=f"qhT{t}", name=f"qhT{t}")
               for t in range(2)]
        khT = [persist.tile([P, L], BF, tag=f"khT{t}", name=f"khT{t}")
               for t in range(2)]
        def emit_qk_proj(src_ap, wsb, dst, cp):
            # cp is a 512-wide L chunk
            xt = stream.tile([P, NDT, 512], BF, tag="xt", name="xt")
            nc.sync.dma_start(
                xt[:], src_ap[:, cp * 512:(cp + 1) * 512].rearrange(
                    "(t p) n -> p t n", p=P))
            acc = ps1.tile([P, 1024], F32, tag="ps1", name="acc")
            for dt in range(NDT):
                for t in range(2):
                    nc.tensor.matmul(
                        acc[:, t * 512:(t + 1) * 512],
                        lhsT=wsb[:, dt, t * P:(t + 1) * P],
                        rhs=xt[:, dt, :],
                        start=(dt == 0), stop=(dt == NDT - 1))
            for t in range(2):
                nc.vector.tensor_copy(
                    dst[t][:, cp * 512:(cp + 1) * 512],
                    acc[:, t * 512:(t + 1) * 512])

        for cp in range(4):
            emit_qk_proj(qT, wq_sb, qhT, cp)
        for cp in range(2):
            emit_qk_proj(kT, wk_sb, khT, cp)

        # ---- v: per-kt strips, loaded just in time ----
        nc.sync.dma_start(wv_sb[:], wv.rearrange("(t p) n -> p t n", p=P))
        nc.sync.dma_start(wfc_sb[:], wfc.rearrange("(t p) n -> p t n", p=P))
        vh_aug = [persist.tile([P, NLT, 66], BF, tag=f"vh{h}", name=f"vh{h}")
                  for h in range(HPG)]
        for h in range(HPG):
            nc.vector.memset(vh_aug[h][:], 1.0)

        def emit_v_proj(kt):
            vs = vstream.tile([P, NDT, P], BF, tag="vs", name="vs")
            nc.sync.dma_start(
                vs[:], vT[:, kt * P:(kt + 1) * P].rearrange(
                    "(t p) n -> p t n", p=P))
            acc = ps1.tile([P, 1024], F32, tag="ps1", name="vacc")
            for dt in range(NDT):
                nc.tensor.matmul(
                    acc[:, 0:HD],
                    lhsT=vs[:, dt, :],
                    rhs=wv_sb[:, dt, :],
                    start=(dt == 0), stop=(dt == NDT - 1))
            for h in range(HPG):
                nc.vector.tensor_copy(vh_aug[h][:, kt, 0:DQK],
                                      acc[:, h * DQK:(h + 1) * DQK])

        # ---- attention ----
        oall = [persist.tile([P, L], BF, tag=f"oall{t}", name=f"oall{t}")
                for t in range(2)]
        oTs = {}
        ets = {}
        stags = {}
        rzbs = {}

        def emit_ST_exp(h, kt):
            th, ph = h // 2, (h % 2) * DQK
            et = etp.tile([P, L], BF, tag="et", name="et")
            for piece in range(2):
                st = ps1.tile([P, 1024], F32, tag="ps1", name="st")
                for c in range(2):
                    off = piece * 1024 + c * 512
                    nc.tensor.matmul(
                        st[:, c * 512:(c + 1) * 512],
                        lhsT=khT[th][ph:ph + DQK, kt * P:(kt + 1) * P],
                        rhs=qhT[th][ph:ph + DQK, off:off + 512],
                        start=True, stop=True)
                nc.scalar.activation(
                    et[:, piece * 1024:(piece + 1) * 1024], st[:],
                    mybir.ActivationFunctionType.Exp, scale=INV_TEMP)
            ets[(h, kt)] = et

        def emit_AV(h, kt):
            oT = oTs[h]
            et = ets[(h, kt)]
            for c in range(4):
                nc.tensor.matmul(
                    oT[0:66, c * 512:(c + 1) * 512],
                    lhsT=vh_aug[h][:, kt, :],
                    rhs=et[:, c * 512:(c + 1) * 512],
                    start=(kt == 0), stop=(kt == NLT - 1))

        def emit_A_tail1(h):
            # one fast PSUM->SBUF copy releases the oT slot for head h+1
            emit_AV(h, NLT - 1)
            stag = zp.tile([66, L], F32, tag="stag", name="stag")
            nc.vector.tensor_copy(stag[:], oTs[h][0:66, :])
            stags[h] = stag
            th, ph = h // 2, (h % 2) * DQK
            nc.vector.tensor_copy(oall[th][ph:ph + DQK, :], stag[0:DQK, :])

        def emit_A_tail2(h):
            th, ph = h // 2, (h % 2) * DQK
            stag = stags.pop(h)
            # rzb[p, q] = 1/Z[q]: reciprocal of the Z row, broadcast to all
            # partitions on the (otherwise idle) GPSIMD engine
            rb_row = zp.tile([1, L], BF, tag="rb_row", name="rb_row")
            with nc.allow_low_precision(reason="1/Z broadcast in bf16"):
                nc.vector.reciprocal(rb_row[:], stag[DQK:DQK + 1, :])
            rzb = zp.tile([P, L], BF, tag="rzb", bufs=2, name="rzb")
            rzbs[h] = rzb
            nc.gpsimd.partition_broadcast(rzb[:], rb_row[:])
            nc.vector.tensor_mul(oall[th][ph:ph + DQK, :],
                                 oall[th][ph:ph + DQK, :],
                                 rzb[ph:ph + DQK, :])

        def emit_B_step(h, kt):
            # attn^T[k, q] = E_T[k, q] / Z[q]; host untransposes
            at = atp.tile([P, L], F32, tag="at", name="at")
            nc.vector.tensor_mul(at[:], ets.pop((h, kt)), rzbs[h][:])
            nc.sync.dma_start(attn_out[h, kt * P:(kt + 1) * P, :], at[:])

        def emit_FC(qt):
            fc_ps = ps1.tile([P, 1024], F32, tag="ps1", name="fc_ps")
            for t in range(2):
                for c in range(2):
                    nc.tensor.matmul(
                        fc_ps[:, c * 512:(c + 1) * 512],
                        lhsT=oall[t][:, qt * P:(qt + 1) * P],
                        rhs=wfc_sb[:, t, c * 512:(c + 1) * 512],
                        start=(t == 0), stop=(t == 1))
            fc_sb = fcp.tile([P, D], BF, tag="fc", name="fc_sb")
            nc.vector.tensor_copy(fc_sb[:], fc_ps[:])
            nc.sync.dma_start(fc_out[qt * P:(qt + 1) * P, :], fc_sb[:])

        for h in range(HPG + 1):
            if h < HPG:
                oTs[h] = psO.tile([P, L], F32, tag="psO", name="oT")
            for s in range(NLT + 3):
                if h < HPG and s < NLT:
                    emit_ST_exp(h, s)
                    if s > 0:
                        emit_AV(h, s - 1)
                if h == 0 and s in (0, 1):
                    emit_qk_proj(kT, wk_sb, khT, 2 + s)
                if h == 0 and s < NLT:
                    emit_v_proj(s)
                if s == 2 and h > 0:
                    emit_A_tail2(h - 1)
                if h > 0 and s >= 3:
                    emit_B_step(h - 1, s - 3)
                if h == HPG and s >= 3:
                    emit_FC(s - 3)
            if h < HPG:
                emit_A_tail1(h)


ROWS = B * L // NCORES  # 512 rows per core in stage 2


def _build_stage2():
    nc = bacc.Bacc("TRN2", target_bir_lowering=False, debug=False,
                   num_devices=NCORES)
    ps = [nc.dram_tensor(f"p{i}", [ROWS, D], BF, kind="ExternalInput").ap()
          for i in range(G)]
    resid = nc.dram_tensor("resid", [ROWS, D], F32, kind="ExternalInput").ap()
    gamma = nc.dram_tensor("gamma", [1, D], F32, kind="ExternalInput").ap()
    beta = nc.dram_tensor("beta", [1, D], F32, kind="ExternalInput").ap()
    out = nc.dram_tensor("out", [ROWS, D], F32, kind="ExternalOutput").ap()

    with tile.TileContext(nc) as tc:
        _stage2_body(tc, ps, resid, gamma, beta, out)
    nc.compile()
    return nc


def _stage2_body(tc, ps, resid, gamma, beta, out):
    nc = tc.nc
    import contextlib
    ctx = contextlib.ExitStack()
    RT = ROWS // P  # 4 row-tiles
    with ctx:
        pool = ctx.enter_context(tc.tile_pool(name="p", bufs=3))
        sing = ctx.enter_context(tc.tile_pool(name="s", bufs=1))
        stat = ctx.enter_context(tc.tile_pool(name="st", bufs=4))

        gam = sing.tile([P, D], F32, tag="gam")
        bet = sing.tile([P, D], F32, tag="bet")
        nc.gpsimd.dma_start(
            gam[:], bass.AP(tensor=gamma.tensor, offset=gamma.offset,
                            ap=[[0, P]] + [list(a) for a in gamma.ap[1:]]))
        nc.gpsimd.dma_start(
            bet[:], bass.AP(tensor=beta.tensor, offset=beta.offset,
                            ap=[[0, P]] + [list(a) for a in beta.ap[1:]]))
        eps_t = sing.tile([P, 1], F32, tag="eps")
        nc.vector.memset(eps_t[:], EPS)

        for rt in range(RT):
            sl = slice(rt * P, (rt + 1) * P)
            r = pool.tile([P, D], F32, tag="r", name="r")
            nc.sync.dma_start(r[:], resid[sl, :])
            pt = [pool.tile([P, D], BF, tag=f"pt{i}", name=f"pt{i}")
                  for i in range(G)]
            for i in range(G):
                nc.sync.dma_start(pt[i][:], ps[i][sl, :])
            s01 = pool.tile([P, D], F32, tag="s01", name="s01")
            s23 = pool.tile([P, D], F32, tag="s23", name="s23")
            nc.vector.tensor_add(s01[:], pt[0][:], pt[1][:])
            nc.vector.tensor_add(s23[:], pt[2][:], pt[3][:])
            nc.vector.tensor_add(s01[:], s01[:], s23[:])
            acc = pool.tile([P, D], F32, tag="acc", name="acc")
            nc.vector.tensor_add(acc[:], s01[:], r[:])
            stt = stat.tile([P, 2, 6], F32, tag="stt", name="stt")
            grp = acc[:].rearrange("p (n d) -> p n d", n=2)
            for sg in range(2):
                nc.vector.bn_stats(stt[:, sg, :], grp[:, sg, :])
            mv = stat.tile([P, 2], F32, tag="mv", name="mv")
            nc.vector.bn_aggr(mv[:], stt[:])
            sd = stat.tile([P, 1], F32, tag="sd", name="sd")
            nc.scalar.activation(sd[:], mv[:, 1:2],
                                 mybir.ActivationFunctionType.Sqrt,
                                 bias=eps_t[:])
            nc.vector.reciprocal(sd[:], sd[:])
            nrm = pool.tile([P, D], F32, tag="nrm", name="nrm")
            nc.vector.tensor_scalar(nrm[:], acc[:], mv[:, 0:1], sd[:],
                                    op0=mybir.AluOpType.subtract,
                                    op1=mybir.AluOpType.mult)
            nc.vector.tensor_mul(nrm[:], nrm[:], gam[:])
            nc.vector.tensor_add(nrm[:], nrm[:], bet[:])
            nc.sync.dma_start(out[sl, :], nrm[:])


_STAGE1 = None
_STAGE2 = None
LAST_TIMES = {}


def _get_stages():
    global _STAGE1, _STAGE2
    if _STAGE1 is None:
        _STAGE1 = _build_stage1()
    if _STAGE2 is None:
        _STAGE2 = _build_stage2()
    return _STAGE1, _STAGE2


def kernel(q, k, v, w_q, w_k, w_v, w_fc, gamma, beta):
    q = np.asarray(q, np.float32)
    k = np.asarray(k, np.float32)
    v = np.asarray(v, np.float32)
    nc1, nc2 = _get_stages()

    qT = np.ascontiguousarray(q.transpose(0, 2, 1)).astype(BF16)
    kT = np.ascontiguousarray(k.transpose(0, 2, 1)).astype(BF16)
    vT = np.ascontiguousarray(v.transpose(0, 2, 1)).astype(BF16)
    wqb = np.asarray(w_q, np.float32).astype(BF16)
    wkb = np.asarray(w_k, np.float32).astype(BF16)
    wvb = np.asarray(w_v, np.float32).astype(BF16)
    wfcb = np.asarray(w_fc, np.float32).astype(BF16)

    in_maps1 = []
    for c in range(NCORES):
        b, g = c // G, c % G
        cs = slice(g * HD, (g + 1) * HD)
        in_maps1.append({
            "qT": qT[b], "kT": kT[b], "vT": vT[b],
            "wq": np.ascontiguousarray(wqb[:, cs]),
            "wk": np.ascontiguousarray(wkb[:, cs]),
            "wv": np.ascontiguousarray(wvb[:, cs]),
            "wfc": np.ascontiguousarray(wfcb[cs, :]),
        })
    import time as _time

    def _run(nc, maps):
        # one retry: a previously wedged NeuronCore can fail the first
        # execution after reset with NRT_EXEC_UNIT_UNRECOVERABLE
        try:
            return run_bass_kernel_spmd(nc, maps, core_ids=list(range(NCORES)))
        except Exception:
            _time.sleep(2.0)
            return run_bass_kernel_spmd(nc, maps, core_ids=list(range(NCORES)))

    _t = _time.perf_counter()
    res1 = _run(nc1, in_maps1)
    LAST_TIMES["stage1_wall"] = _time.perf_counter() - _t

    attn = np.empty((B, H, L, L), np.float32)
    for c in range(NCORES):
        b, g = c // G, c % G
        # device writes attn^T per head; untranspose while unsharding
        attn[b, g * HPG:(g + 1) * HPG] = \
            res1.results[c]["attn_part"].swapaxes(1, 2)

    qr = q.reshape(B * L, D)
    gam = np.asarray(gamma, np.float32).reshape(1, D)
    bet = np.asarray(beta, np.float32).reshape(1, D)
    in_maps2 = []
    for c in range(NCORES):
        b, r = c // G, c % G
        rows = slice(r * ROWS, (r + 1) * ROWS)
        in_maps2.append({
            **{f"p{i}": res1.results[b * G + i]["fc_part"][rows]
               for i in range(G)},
            "resid": qr[b * L:(b + 1) * L][rows],
            "gamma": gam, "beta": bet,
        })
    _t = _time.perf_counter()
    res2 = _run(nc2, in_maps2)
    LAST_TIMES["stage2_wall"] = _time.perf_counter() - _t

    o = np.empty((B * L, D), np.float32)
    for c in range(NCORES):
        b, r = c // G, c % G
        o[b * L + r * ROWS: b * L + (r + 1) * ROWS] = res2.results[c]["out"]
    return o.reshape(B, L, D), attn
